# revision 33
# baseline (speedup 1.0000x reference)
"""MoE (BruteForceMoELinear) Trainium2 kernel — bf16 expert-parallel.

Strategy: expert-parallel across 8 NeuronCores.  The host dispatches
token rows by `gate_idx` (stable sort), folds the per-row gate score
into the activations (scores >= 0 commute through ReLU), pads each
expert's batch to capacity C, and hands core e bf16-packed operands.

Per-core compute: y_e^T = W2_e @ relu(W1_e @ x_e^T), bf16 matmuls with
fp32 PSUM accumulation.  Tokens split into a big chunk A (<=512 cols)
and a small remainder B.  GEMM1 opens ko-major over the first FO1
f-groups so the PE consumes each (W1-ko, x-ko) row-DMA the moment it
lands; W1-ko and x-ko are packed into a single DRAM row per ko to
minimize per-DMA descriptor-generation serialization.  The rest runs
fo-major against streamed W1, with B's tiny groups woven between A
groups.  GEMM2 ends with a column-split last d-group so the final
evict+DMA tail is short.  PSUM evictions alternate between the scalar
and vector engines.
"""

import numpy as np
import ml_dtypes

import os

NUM_EXPERT = 8
N_CORES = 8
P = 128
FO1 = int(os.environ.get("K_FO1", "6"))  # ko-major head fo-groups
_CUT = int(os.environ.get("K_CUT", "2"))     # W1 cols in first DMA piece
_SUBS = os.environ.get("K_SUBS", "12,4")     # last d-group col split /16
_LASTACT = int(os.environ.get("K_LASTACT", "1"))  # 1: evict subs Act-first

_CACHE = {}


def _chunks_for(C):
    if C <= 512:
        return [C]
    assert C <= 1024
    return [512, C - 512]


def _build(C, KO, FO, repeat=1):
    key = (C, KO, FO, repeat)
    if key in _CACHE:
        return _CACHE[key]

    import concourse.mybir as mybir
    import concourse.tile as tile
    from concourse import bacc

    f32 = mybir.dt.float32
    bf16 = mybir.dt.bfloat16
    chunks = _chunks_for(C)
    TA = chunks[0]
    TB = chunks[1] if len(chunks) > 1 else 0
    nfo1 = min(FO1, FO)
    FOB = FO - nfo1
    RS = TA + nfo1 * P           # row stride: x-ko | w1a-ko
    XWN = KO * RS + KO * TB      # + xB appended at the end

    nc = bacc.Bacc("TRN2", target_bir_lowering=False, debug=False,
                   num_devices=N_CORES)

    xw = nc.dram_tensor("xw", (P, XWN), bf16, kind="ExternalInput")
    w1b = nc.dram_tensor("w1b", (P, FOB, KO * P), bf16, kind="ExternalInput")
    w2 = nc.dram_tensor("w2", (P, KO, FO * P), bf16, kind="ExternalInput")
    yt = nc.dram_tensor("yt", (P, KO * C), bf16, kind="ExternalOutput")

    with tile.TileContext(nc) as tc:
        with tc.tile_pool(name="wpool", bufs=1) as wpool, \
             tc.tile_pool(name="ypool", bufs=4) as ypool, \
             tc.tile_pool(name="psA", bufs=6, space="PSUM") as psA, \
             tc.tile_pool(name="psB", bufs=2, space="PSUM") as psB:

            xwsb = wpool.tile([P, XWN], bf16, name="xwsb")
            w1bsb = (wpool.tile([P, FOB, KO * P], bf16, name="w1bsb")
                     if FOB else None)
            w2sb = wpool.tile([P, KO, FO * P], bf16, name="w2sb")
            hA = wpool.tile([P, FO, TA], bf16, name="hA")
            hB = wpool.tile([P, FO, TB], bf16, name="hB") if TB else None

            def xA_ap(ko):
                return xwsb[:, ko * RS:ko * RS + TA]

            def xB_ap(ko):
                return xwsb[:, KO * RS + ko * TB:KO * RS + (ko + 1) * TB]

            def w1_ap(f, ko):
                if f < nfo1:
                    off = ko * RS + TA + f * P
                    return xwsb[:, off:off + P]
                return w1bsb[:, f - nfo1, ko * P:(ko + 1) * P]

            # --- DMAs: emission order == consumption order -------------
            cut = TA + _CUT * P if nfo1 >= _CUT else RS
            nc.sync.dma_start(xwsb[:, 0:cut], xw.ap()[:, 0:cut])
            if cut < RS:
                nc.sync.dma_start(xwsb[:, cut:RS], xw.ap()[:, cut:RS])
            for ko in range(1, KO):
                hi = (ko + 1) * RS if ko < KO - 1 else XWN
                nc.sync.dma_start(xwsb[:, ko * RS:hi], xw.ap()[:, ko * RS:hi])
            fo = 0
            while fo < FOB:
                hi = min(fo + int(os.environ.get("K_W1B", "4")), FOB)
                nc.sync.dma_start(w1bsb[:, fo:hi, :], w1b.ap()[:, fo:hi, :])
                fo = hi
            nc.sync.dma_start(w2sb[:, 0:2, :], w2.ap()[:, 0:2, :])
            nc.sync.dma_start(w2sb[:, 2:KO, :], w2.ap()[:, 2:KO, :])

            def evict1(dst, src, use_act):
                if use_act:
                    nc.scalar.activation(dst, src,
                                         mybir.ActivationFunctionType.Relu)
                else:
                    nc.vector.tensor_scalar_max(dst, src, 0.0)

            # Keep-warm bridge: the cost model resets the PE p-state
            # anchor when the PE idles more than ~0.8us, and the ramp to
            # full rate takes 3us from the anchor.  Emit a chain of tiny
            # matmuls, each gated by a ~0.6us Pool-engine memset, so PE
            # activity recurs every <0.7us until the first real matmul
            # (~3.5us, after the row-0 DMA) — which then runs full-rate.
            warm = wpool.tile([P, 16], bf16, name="warm")
            NPACE = 18
            pace = wpool.tile([P, (NPACE + 1) * 16], bf16, name="pace")
            bconst = nc.const_aps.aps[(mybir.dt.bfloat16, 1.0)]
            nc.tensor.ldweights(bconst)
            nc.vector.memset(warm[:], 0.0)
            nc.vector.memset(pace[:, 0:16], 0.0)
            wps = psB.tile([P, 16], f32, name="wps", tag="pB")
            nc.tensor.matmul(wps[0:16, :], warm[:], warm[:],
                             start=True, stop=True)
            for k in range(NPACE):
                nc.vector.tensor_scalar_add(
                    pace[:, (k + 1) * 16:(k + 2) * 16],
                    pace[:, k * 16:(k + 1) * 16], 0.0)
                nc.tensor.matmul(
                    wps[0:16, :], warm[:],
                    pace[:, (k + 1) * 16:(k + 2) * 16],
                    start=True, stop=True)

            # --- GEMM1 phase 1: ko-major over fo 0..nfo1 on chunk A ----
            p1s = [psA.tile([P, TA], f32, name=f"p1f{f}", tag="pA")
                   for f in range(nfo1)]
            for ko in range(KO):
                for f in range(nfo1):
                    nc.tensor.matmul(p1s[f][:], w1_ap(f, ko), xA_ap(ko),
                                     start=(ko == 0), stop=(ko == KO - 1))
                    if ko == KO - 1:
                        evict1(hA[:, f, :], p1s[f][:], f % 2 == 0)

            # --- GEMM1 phase 2: fo-major, B's groups interleaved -------
            def gemm1B(f):
                pb = psB.tile([P, TB], f32, name="pb", tag="pB")
                for ko in range(KO):
                    nc.tensor.matmul(pb[:], w1_ap(f, ko), xB_ap(ko),
                                     start=(ko == 0), stop=(ko == KO - 1))
                nc.vector.tensor_scalar_max(hB[:, f, :], pb[:], 0.0)

            bq = list(range(FO)) if TB else []
            NB = len(bq)
            nA2 = max(FO - nfo1, 1)
            for i, f in enumerate(range(nfo1, FO)):
                p1 = psA.tile([P, TA], f32, name="p1", tag="pA")
                for ko in range(KO):
                    nc.tensor.matmul(p1[:], w1_ap(f, ko), xA_ap(ko),
                                     start=(ko == 0), stop=(ko == KO - 1))
                evict1(hA[:, f, :], p1[:], True)
                ntake = ((i + 1) * NB) // nA2 - (i * NB) // nA2
                for _ in range(ntake):
                    gemm1B(bq.pop(0))
            for f in bq:
                gemm1B(f)

            # --- GEMM2 --------------------------------------------------
            def gemm2A(do, c0, c1, ysb, use_act=True, dma_eng=None):
                p2 = psA.tile([P, TA], f32, name="p2", tag="pA")
                for f in range(FO):
                    nc.tensor.matmul(p2[:, 0:c1 - c0],
                                     w2sb[:, do, f * P:(f + 1) * P],
                                     hA[:, f, c0:c1],
                                     start=(f == 0), stop=(f == FO - 1))
                if use_act:
                    nc.scalar.copy(ysb[:], p2[:, 0:c1 - c0])
                else:
                    nc.vector.tensor_scalar_add(ysb[:], p2[:, 0:c1 - c0], 0.0)
                (dma_eng or nc.sync).dma_start(
                    yt.ap()[:, do * TA + c0:do * TA + c1], ysb[:])

            def gemm2B(do, ysbB):
                pb = psB.tile([P, TB], f32, name="p2b", tag="pB")
                for f in range(FO):
                    nc.tensor.matmul(pb[:],
                                     w2sb[:, do, f * P:(f + 1) * P],
                                     hB[:, f, :],
                                     start=(f == 0), stop=(f == FO - 1))
                nc.vector.tensor_scalar_add(
                    ysbB[:, do * TB:(do + 1) * TB], pb[:], 0.0)

            ysbB = (ypool.tile([P, KO * TB], bf16, tag="yB", name="yB")
                    if TB else None)
            for do in range(KO - 1):
                ysb = ypool.tile([P, TA], bf16, tag="yA", name="yA")
                gemm2A(do, 0, TA, ysb)
                if TB:
                    gemm2B(do, ysbB)
            if TB:
                gemm2B(KO - 1, ysbB)
                nc.sync.dma_start(yt.ap()[:, KO * TA:KO * C], ysbB[:])

            # last A d-group, column-split with decreasing subgroups so
            # the terminal output DMAs stay spaced >= the HWDGE fixed
            # cost and the final evict+DMA covers few columns.
            if TA >= 512:
                subs = [TA * int(v) // 16 for v in _SUBS.split(",")]
            else:
                subs = [TA]
            c0 = 0
            for s, sub in enumerate(subs):
                ysb = ypool.tile([P, sub], bf16, tag="yA3", name="yA3")
                gemm2A(KO - 1, c0, c0 + sub, ysb,
                       use_act=(s % 2 == 0) == bool(_LASTACT))
                c0 += sub

    nc.compile()
    _CACHE[key] = nc
    return nc


_last = {}


def _pack_inputs(xs, w_htoh4, w_h4toh, idx_split, C, KO, FO):
    bf16 = ml_dtypes.bfloat16
    chunks = _chunks_for(C)
    TA = chunks[0]
    TB = chunks[1] if len(chunks) > 1 else 0
    nfo1 = min(FO1, FO)
    RS = TA + nfo1 * P
    d_model = KO * P
    in_maps = []
    for e in range(NUM_EXPERT):
        idx = idx_split[e]
        cnt = len(idx)
        xT = np.zeros((d_model, C), dtype=np.float32)
        if cnt:
            xT[:, :cnt] = xs[idx].T
        xk = xT.reshape(KO, P, C)                          # [ko, p, c]
        w1t = w_htoh4[e].T.reshape(KO, P, FO, P)          # [ko, p, fo, f]
        rows = []
        for ko in range(KO):
            rows.append(xk[ko, :, :TA])                   # x-ko  (P, TA)
            rows.append(w1t[ko, :, :nfo1, :].reshape(P, nfo1 * P))
        xw_h = np.concatenate(rows, axis=1)               # (P, KO*RS)
        if TB:
            xB = xk[:, :, TA:C].transpose(1, 0, 2).reshape(P, KO * TB)
            xw_h = np.concatenate([xw_h, xB], axis=1)
        w1b_h = w1t[:, :, nfo1:, :].transpose(1, 2, 0, 3) \
            .reshape(P, FO - nfo1, KO * P)
        w2t = w_h4toh[e].T.reshape(FO, P, KO, P)          # [fo, p, do, d]
        w2_h = w2t.transpose(1, 2, 0, 3).reshape(P, KO, FO * P)
        in_maps.append({
            "xw": np.ascontiguousarray(xw_h.astype(bf16)),
            "w1b": np.ascontiguousarray(w1b_h.astype(bf16)),
            "w2": np.ascontiguousarray(w2_h.astype(bf16)),
        })
    return in_maps


def kernel(inp, gate_idx, gate_score, w_htoh4, w_h4toh):
    inp = np.ascontiguousarray(np.asarray(inp, dtype=np.float32))
    gate_idx = np.asarray(gate_idx)
    gate_score = np.asarray(gate_score, dtype=np.float32)
    w_htoh4 = np.asarray(w_htoh4, dtype=np.float32)
    w_h4toh = np.asarray(w_h4toh, dtype=np.float32)

    B, d_model = inp.shape
    n_expert, d_ff, _ = w_htoh4.shape
    assert n_expert == NUM_EXPERT
    KO = d_model // P
    FO = d_ff // P

    gi = gate_idx.astype(np.int64)
    order = np.argsort(gi, kind="stable")
    counts = np.bincount(gi, minlength=NUM_EXPERT)
    idx_split = np.split(order, np.cumsum(counts)[:-1])

    C = max(int(-(-counts.max() // 16) * 16), 256)
    TA = _chunks_for(C)[0]

    scores_flat = gate_score.reshape(-1)
    xs = inp * scores_flat[:, None]

    nc = _build(C, KO, FO)
    in_maps = _pack_inputs(xs, w_htoh4, w_h4toh, idx_split, C, KO, FO)

    from concourse import bass_utils
    res = bass_utils.run_bass_kernel_spmd(nc, in_maps,
                                          core_ids=list(range(N_CORES)))

    _last.update(nc=nc, in_maps=in_maps, res=res, C=C, KO=KO, FO=FO)

    y_full = np.empty((B, d_model), dtype=np.float32)
    for e in range(NUM_EXPERT):
        idx = idx_split[e]
        if len(idx) == 0:
            continue
        yt_h = res.results[e]["yt"].astype(np.float32)  # (P, KO*C)
        yA = yt_h[:, :KO * TA].reshape(P, KO, TA)
        if C > TA:
            yB = yt_h[:, KO * TA:].reshape(P, KO, C - TA)
            yk = np.concatenate([yA, yB], axis=2)
        else:
            yk = yA
        yT = yk.transpose(1, 0, 2).reshape(d_model, C)
        y_full[idx] = yT[:, :len(idx)].T
    out = y_full[0::2] + y_full[1::2]
    return np.ascontiguousarray(out, dtype=np.float32)
